# revision 22
# baseline (speedup 1.0000x reference)
"""Trainium2 Bass kernel for nn_DecoderTree (goal-driven tree decoder).

Strategy: data-parallel over batch B=48 -> 8 cores x 6 lanes. The teacher-forced
23-step recurrence is re-scheduled into dependency waves (2 nodes per lane per
wave); the merge/term subsystem (independent of the goal recurrence) is leveled
into rounds and interleaved. All per-core variation (gather indices, masks,
bias rows, arena init) is input DATA, so one SPMD program serves all 8 cores.
State rows live in DRAM arenas; gathers use indirect DMA; gathered rows are
transposed on the PE into feature-on-partition layout for the gate matmuls.
Sigmoid is computed as 0.5+0.5*tanh(x/2) to stay in the exp/tanh ACT table set.
"""
import os
import sys
import numpy as np
import ml_dtypes

for _p in ('/opt/trn_rl_repo',):
    if _p not in sys.path:
        sys.path.insert(0, _p)

BF16 = ml_dtypes.bfloat16

H, E, NUM_START, N_CONST, N_NUM = 512, 128, 5, 2, 8
B, S, T, DE = 48, 256, 23, 768
O = N_CONST + N_NUM
NCORE, BL = 8, 6
HC = H // 128
KE = DE // 128
CAP = 2


# ============================ planning =====================================

def build_plan(tgt):
    Tt, Bs = tgt.shape
    cur_idx = np.zeros((Tt, Bs), np.int32)
    left_src = np.zeros((Tt, Bs), np.int32)
    left_valid = np.zeros((Tt, Bs), bool)
    node = [[1] for _ in range(Bs)]
    embs = [[] for _ in range(Bs)]
    lc = [(False, 0)] * Bs
    merges = []
    for t in range(Tt):
        for i in range(Bs):
            left_valid[t, i], left_src[t, i] = lc[i]
            cur_idx[t, i] = node[i][-1] if node[i] else 0
        step = [[] for _ in range(Bs)]
        for i in range(Bs):
            if not node[i]:
                lc[i] = (False, 0)
                continue
            node[i].pop()
            tok = int(tgt[t, i])
            if tok < NUM_START:
                node[i].append(3 + 2 * t)
                node[i].append(2 + 2 * t)
                embs[i].append(('op', t))
            else:
                while embs[i] and embs[i][-1][0] == 'term':
                    term = embs[i].pop()
                    opn = embs[i].pop()
                    step[i].append((opn[1], term[1]))
                embs[i].append(('term', t))
            lc[i] = (True, embs[i][-1][1]) if (embs[i] and embs[i][-1][0] == 'term') else (False, 0)
        md = max(len(m) for m in step)
        depth = []
        for d in range(md):
            opA = np.zeros(Bs, np.int32)
            tmA = np.zeros(Bs, np.int32)
            ac = np.zeros(Bs, bool)
            for i in range(Bs):
                if len(step[i]) > d:
                    opA[i], tmA[i] = step[i][d]
                    ac[i] = True
            depth.append((opA, tmA, ac))
        merges.append(depth)
    gen_input = np.where(tgt < NUM_START, tgt, 0).astype(np.int32)
    num_idx = np.clip(tgt - NUM_START, 0, None).astype(np.int32)
    is_term = tgt >= NUM_START
    return cur_idx, left_src, left_valid, gen_input, num_idx, merges, is_term


class Schedule:
    pass


def make_schedule(y):
    tgt = np.asarray(y).T.astype(np.int64)
    cur_idx, left_src, left_valid, gen_input, num_idx, merges, is_term = build_plan(tgt)

    parent = np.full((T, B), -1, np.int64)
    side = np.zeros((T, B), np.int64)
    for t in range(T):
        for b in range(B):
            ci = cur_idx[t, b]
            if ci >= 2:
                parent[t, b] = (ci - 2) // 2
                side[t, b] = (ci - 2) % 2

    lvl = np.zeros((T, B), np.int64)
    mop_of = {}
    for t in range(T):
        for b in range(B):
            if not is_term[t, b]:
                continue
            cur_lvl = 0
            lst = []
            for d, (opA, tmA, ac) in enumerate(merges[t]):
                if ac[b]:
                    cur_lvl = max(cur_lvl, lvl[tmA[b], b]) + 1
                    lst.append((d, cur_lvl, int(tmA[b]), int(opA[b])))
            lvl[t, b] = cur_lvl
            mop_of[(t, b)] = lst

    children = [[[] for _ in range(T)] for _ in range(B)]
    for t in range(T):
        for b in range(B):
            if parent[t, b] >= 0:
                children[b][parent[t, b]].append(t)
    cp = np.zeros((T, B), np.int64)
    for b in range(B):
        for t in range(T - 1, -1, -1):
            cp[t, b] = 1 + max([cp[c, b] for c in children[b][t]], default=0)

    wave = np.full((T, B), -1, np.int64)
    for b in range(B):
        done = np.zeros(T, bool)
        w = 0
        while not done.all():
            ready = []
            for t in range(T):
                if done[t]:
                    continue
                p = parent[t, b]
                if p >= 0 and not (wave[p, b] >= 0 and wave[p, b] < w):
                    continue
                if left_valid[t, b] and lvl[left_src[t, b], b] > w:
                    continue
                ready.append(t)
            ready.sort(key=lambda t: -cp[t, b])
            for t in ready[:CAP]:
                wave[t, b] = w
                done[t] = True
            w += 1
            assert w < 64, "schedule runaway"
    NW = int(wave.max() + 1)

    nodes_wl = {}
    for t in range(T):
        for b in range(B):
            nodes_wl.setdefault((int(wave[t, b]), b), []).append(t)
    G = np.zeros(NW, np.int64)
    for (w, b), lst in nodes_wl.items():
        lst.sort(key=lambda t: -cp[t, b])
        G[w] = max(G[w], len(lst))
    M = (BL * G).astype(np.int64)

    NR = int(lvl.max())
    round_ops = [[[] for _ in range(NCORE)] for _ in range(NR + 1)]
    for (t, b), lst in mop_of.items():
        for (d, L, tm_s, op_s) in lst:
            round_ops[L][b // BL].append((t, b, d, tm_s, op_s))
    MM = np.zeros(NR + 1, np.int64)
    for L in range(1, NR + 1):
        MM[L] = max(len(round_ops[L][c]) for c in range(NCORE))

    # arena_c: slot 0 zero, 1..BL summaries, then per-wave cand slots (l then r)
    pos = 1 + BL
    cand_base = []
    for w in range(NW):
        cand_base.append(pos)
        pos += 2 * int(M[w])
    NSLOT_C = pos
    # arena_t: slot 0 zero, 1..BL*T term inits, then merge round outputs
    tpos = 1 + BL * T
    round_base = [0] * (NR + 1)
    for L in range(1, NR + 1):
        round_base[L] = tpos
        tpos += int(MM[L])
    NSLOT_T = tpos

    goff = np.concatenate([[0], np.cumsum(G)]).astype(np.int64)
    NPL = int(goff[NW])
    NQ = BL * NPL

    sch = Schedule()
    sch.__dict__.update(dict(
        tgt=tgt, cur_idx=cur_idx, left_src=left_src, left_valid=left_valid,
        gen_input=gen_input, num_idx=num_idx, merges=merges, is_term=is_term,
        parent=parent, side=side, lvl=lvl, mop_of=mop_of, wave=wave, NW=NW,
        nodes_wl=nodes_wl, G=G, M=M, NR=NR, round_ops=round_ops, MM=MM,
        cand_base=cand_base, round_base=round_base,
        NSLOT_C=NSLOT_C, NSLOT_T=NSLOT_T,
        goff=goff, NPL=NPL, NQ=NQ,
    ))

    node_col = np.full((T, B), -1, np.int64)
    node_pos = np.full((T, B), -1, np.int64)
    for (w, b), lst in nodes_wl.items():
        lane = b % BL
        for g, t in enumerate(lst):
            node_col[t, b] = lane * G[w] + g
            node_pos[t, b] = goff[w] + g

    def cand_slot(s, b, sd):
        w_p = int(wave[s, b])
        return cand_base[w_p] + sd * int(M[w_p]) + int(node_col[s, b])

    mop_out = {}
    for L in range(1, NR + 1):
        for c in range(NCORE):
            for k, (t, b, d, tm_s, op_s) in enumerate(round_ops[L][c]):
                mop_out[(t, b, d)] = round_base[L] + k
    term_init = lambda lane, t: 1 + lane * T + t
    term_slot = np.zeros((T, B), np.int64)
    for t in range(T):
        for b in range(B):
            lane = b % BL
            lst = mop_of.get((t, b), [])
            term_slot[t, b] = mop_out[(t, b, lst[-1][0])] if lst else term_init(lane, t)

    sch.node_col, sch.node_pos = node_col, node_pos
    sch.term_slot = term_slot
    sch.mop_out = mop_out
    sch.cand_slot = cand_slot
    sch.term_init = term_init
    sch.maxM = int(M.max())
    sch.maxMM = int(MM.max()) if NR > 0 else 1
    return sch


def sched_meta(sch, params):
    p = params
    return dict(
        NW=sch.NW, NR=sch.NR,
        G=[int(v) for v in sch.G], M=[int(v) for v in sch.M],
        MM=[int(v) for v in sch.MM],
        maxM=sch.maxM, maxMM=sch.maxMM,
        NSLOT_C=sch.NSLOT_C, NSLOT_T=sch.NSLOT_T,
        cand_base=list(sch.cand_base), round_base=list(sch.round_base),
        goff=[int(v) for v in sch.goff], NPL=sch.NPL,
        bv_attn=float(np.asarray(p['bv_attn']).reshape(-1)[0]),
        bv_score=float(np.asarray(p['bv_score']).reshape(-1)[0]),
        pred_bias_zero=bool(
            not np.any(np.asarray(p['b_cl'])) and not np.any(np.asarray(p['b_clg']))
            and not np.any(np.asarray(p['b_cr'])) and not np.any(np.asarray(p['b_crg']))),
    )


# ============================ host prep ====================================

def host_prep(sch, core, x, xnm, xnp, encoder_embed, encoder_summary, params):
    p = {k: np.asarray(v, np.float32) for k, v in params.items()}
    bsl = slice(core * BL, (core + 1) * BL)
    x_c = np.asarray(x)[bsl]
    xnm_c = np.asarray(xnm)[bsl]
    xnp_c = np.asarray(xnp)[bsl]
    enc_c = np.asarray(encoder_embed, np.float32)[bsl]
    encsum_c = np.asarray(encoder_summary, np.float32)[bsl]

    out = {}
    out['encT'] = enc_c.reshape(BL * S, DE).T.copy().astype(BF16)
    out['W_ei'] = p['W_ei'].astype(BF16)
    out['WA2'] = p['W_attn'][H:].copy().astype(BF16)
    out['Wq'] = p['W_attn'][:H].copy().astype(BF16)
    out['W_cl'] = p['W_cl'].astype(BF16)
    out['W_clg'] = p['W_clg'].astype(BF16)
    out['W_cr'] = p['W_cr'].astype(BF16)
    out['W_crg'] = p['W_crg'].astype(BF16)
    out['Wg'] = np.concatenate([p['W_gl'][:2 * H], p['W_glg'][:2 * H],
                                p['W_gr'][:2 * H], p['W_grg'][:2 * H]], 1).astype(BF16)
    out['Wm'] = np.concatenate([p['W_m'][E:], p['W_mg'][E:]], 1).astype(BF16)
    out['Ws1'] = p['W_score'][:2 * H].copy().astype(BF16)
    out['W_ops'] = p['W_ops'].astype(BF16)

    vT = np.zeros((128, HC, BL, BL), np.float32)
    vsT = np.zeros((128, HC, BL, BL), np.float32)
    for c in range(HC):
        for lane in range(BL):
            vT[:, c, lane, lane] = p['v_attn'][c * 128:(c + 1) * 128, 0]
            vsT[:, c, lane, lane] = p['v_score'][c * 128:(c + 1) * 128, 0]
    out['vT'] = vT.reshape(128, -1).astype(BF16)
    out['vsT'] = vsT.reshape(128, -1).astype(BF16)
    out['identity'] = np.eye(128, dtype=BF16)
    out['identity_f'] = np.eye(128, dtype=np.float32)

    emb_c = np.tanh(enc_c @ p['W_ei'] + p['b_ei'])
    summary_c = np.tanh(encsum_c @ p['W_si'] + p['b_si'])
    num_emb = emb_c[np.arange(BL)[:, None], xnp_c] * xnm_c[:, :, None]
    all_nums = np.concatenate(
        [np.broadcast_to(p['const_embed'][None], (BL, N_CONST, H)), num_emb], 1)
    num_mask = np.concatenate([np.zeros((BL, N_CONST), bool), ~xnm_c], 1)
    nums_pre = all_nums @ p['W_score'][2 * H:] + p['b_score']
    npt = np.zeros((128, HC, BL, O), np.float32)
    for c in range(HC):
        npt[:, c] = nums_pre.transpose(2, 0, 1)[c * 128:(c + 1) * 128]
    out['nums_preT'] = npt.reshape(128, -1).astype(BF16)

    out['mask_bias'] = np.where(x_c == 0, np.float32(-1e12), np.float32(0)).astype(np.float32)

    def bT(vec):
        return vec.reshape(HC, 128).T.copy()
    out['pred_biasT'] = np.stack(
        [bT(p['b_cl']), bT(p['b_clg']), bT(p['b_cr']), bT(p['b_crg'])], 1
    ).reshape(128, 4 * HC).astype(np.float32)
    out['b_ops'] = p['b_ops'].reshape(NUM_START, 1).astype(np.float32)
    out['A_biasT'] = bT(p['b_attn']).astype(np.float32)
    out['b_ei_T'] = bT(p['b_ei']).astype(np.float32)
    out['b_ei_row'] = p['b_ei'].reshape(1, H).astype(BF16)

    # full-size arena init blobs (zeros beyond the host-filled slots)
    ac_init = np.zeros((sch.NSLOT_C, H), np.float32)
    for lane in range(BL):
        ac_init[1 + lane] = summary_c[lane]
    out['arena_c_init'] = ac_init.astype(BF16)
    at_init = np.zeros((sch.NSLOT_T, H), np.float32)
    for lane in range(BL):
        b = core * BL + lane
        for t in range(T):
            if sch.is_term[t, b]:
                at_init[sch.term_init(lane, t)] = all_nums[lane, sch.num_idx[t, b]]
    out['arena_t_init'] = at_init.astype(BF16)

    NW, G, M, maxM = sch.NW, sch.G, sch.M, sch.maxM
    cidx = np.zeros((maxM, NW), np.int32)
    lidx = np.zeros((maxM, NW), np.int32)
    lv_mask = np.zeros((NW, maxM), np.float32)
    gen_bias = np.zeros((NW, maxM, 4 * H), np.float32)
    for w in range(NW):
        for lane in range(BL):
            b = core * BL + lane
            lst = sch.nodes_wl.get((w, b), [])
            for g in range(int(G[w])):
                col = lane * int(G[w]) + g
                if g < len(lst):
                    t = lst[g]
                    ci = sch.cur_idx[t, b]
                    if ci == 1:
                        cidx[col, w] = 1 + lane
                    elif ci >= 2:
                        cidx[col, w] = sch.cand_slot(int(sch.parent[t, b]), b, int(sch.side[t, b]))
                    if sch.left_valid[t, b]:
                        lidx[col, w] = sch.term_slot[sch.left_src[t, b], b]
                        lv_mask[w, col] = 1.0
                    tok = int(sch.gen_input[t, b])
                    lab = p['op_embed'][tok]
                    for gi, nm in enumerate(['gl', 'glg', 'gr', 'grg']):
                        gen_bias[w, col, gi * H:(gi + 1) * H] = lab @ p['W_' + nm][2 * H:] + p['b_' + nm]
    out['cidx'] = cidx
    out['lidx'] = lidx
    lvm = np.zeros((NW, 128, HC * maxM), np.float32)
    for w in range(NW):
        lvm[w] = np.tile(lv_mask[w][None, :], (128, HC)).reshape(128, HC * maxM)
    out['lv_maskT'] = lvm.reshape(NW * 128, HC * maxM).astype(BF16)
    out['gen_bias'] = gen_bias.reshape(NW * maxM, 4 * H).astype(BF16)

    NR, MM, maxMM = sch.NR, sch.MM, sch.maxMM
    mg_idx = np.zeros((maxMM, 2 * max(NR, 1)), np.int32)
    m_ac = np.zeros((max(NR, 1), maxMM, 1), np.float32)
    m_bias = np.zeros((max(NR, 1), maxMM, 2 * H), np.float32)
    for L in range(1, NR + 1):
        for k, (t, b, d, tm_s, op_s) in enumerate(sch.round_ops[L][core]):
            lane = b % BL
            mg_idx[k, L - 1] = sch.term_slot[tm_s, b]  # tm
            lst = sch.mop_of[(t, b)]
            di = [q[0] for q in lst].index(d)
            mg_idx[k, max(NR, 1) + L - 1] = (sch.term_init(lane, t) if di == 0
                                             else sch.mop_out[(t, b, lst[di - 1][0])])
            m_ac[L - 1, k, 0] = 1.0
            tok = int(sch.tgt[op_s, b])
            lab = p['op_embed'][tok]
            m_bias[L - 1, k, :H] = lab @ p['W_m'][:E] + p['b_m']
            m_bias[L - 1, k, H:] = lab @ p['W_mg'][:E] + p['b_mg']
    out['mgather_idx'] = mg_idx
    out['m_ac'] = m_ac.reshape(max(NR, 1), maxMM).T.copy().astype(np.float32)
    out['m_bias'] = m_bias.reshape(max(NR, 1) * maxMM, 2 * H).astype(BF16)

    host = dict(num_mask=num_mask, node_pos=sch.node_pos[:, bsl].copy())
    return out, host


def assemble_output(sch, core_outs, core_hosts):
    outs = np.zeros((B, T, NUM_START + O), np.float32)
    NPL = sch.NPL
    for core in range(NCORE):
        co = core_outs[core]
        host = core_hosts[core]
        node_pos = host['node_pos']
        ops_out = np.asarray(co['ops_out'], np.float32)
        ns_out = np.asarray(co['ns_out'], np.float32)
        num_mask = host['num_mask']
        for lane in range(BL):
            b = core * BL + lane
            for t in range(T):
                pos = int(node_pos[t, lane])
                outs[b, t, :NUM_START] = ops_out[:, lane * NPL + pos]
                ns = ns_out[lane, pos * O:(pos + 1) * O].copy()
                ns = np.where(num_mask[lane], np.float32(-1e12), ns)
                outs[b, t, NUM_START:] = ns
    return outs


# ============================ device program ================================

def emit_program(meta):
    import concourse.bass as bass
    import concourse.bacc as bacc
    import concourse.tile as tile
    from concourse import mybir

    dt = mybir.dt
    AF = mybir.ActivationFunctionType
    ALU = mybir.AluOpType
    AX = mybir.AxisListType
    BF = dt.bfloat16
    F32 = dt.float32

    NW, NR = meta['NW'], meta['NR']
    G, M, MM = meta['G'], meta['M'], meta['MM']
    maxM, maxMM = meta['maxM'], meta['maxMM']
    NSLOT_C, NSLOT_T = meta['NSLOT_C'], meta['NSLOT_T']
    cand_base, round_base = meta['cand_base'], meta['round_base']
    goff, NPL = meta['goff'], meta['NPL']
    NQ = BL * NPL
    bv_attn, bv_score = meta['bv_attn'], meta['bv_score']
    pred_bias_zero = meta['pred_bias_zero']
    NRX = max(NR, 1)

    nc = bacc.Bacc("TRN2", target_bir_lowering=False, debug=False)

    def din(name, shape, dty=BF):
        return nc.dram_tensor(name, list(shape), dty, kind="ExternalInput")

    d = {}
    d['encT'] = din('encT', (DE, BL * S))
    for nm, shp in [('W_ei', (DE, H)), ('WA2', (H, H)), ('Wq', (H, H)),
                    ('W_cl', (H, H)), ('W_clg', (H, H)),
                    ('W_cr', (2 * H, H)), ('W_crg', (2 * H, H)),
                    ('Wg', (2 * H, 4 * H)), ('Wm', (2 * H, 2 * H)),
                    ('Ws1', (2 * H, H)), ('W_ops', (2 * H, NUM_START))]:
        d[nm] = din(nm, shp)
    d['vT'] = din('vT', (128, HC * BL * BL))
    d['vsT'] = din('vsT', (128, HC * BL * BL))
    d['identity'] = din('identity', (128, 128))
    d['identity_f'] = din('identity_f', (128, 128), F32)
    d['nums_preT'] = din('nums_preT', (128, HC * BL * O))
    d['mask_bias'] = din('mask_bias', (BL, S), F32)
    d['pred_biasT'] = din('pred_biasT', (128, 4 * HC), F32)
    d['b_ops'] = din('b_ops', (NUM_START, 1), F32)
    d['A_biasT'] = din('A_biasT', (128, HC), F32)
    d['b_ei_T'] = din('b_ei_T', (128, HC), F32)
    d['b_ei_row'] = din('b_ei_row', (1, H))
    d['arena_c_init'] = din('arena_c_init', (NSLOT_C, H))
    d['arena_t_init'] = din('arena_t_init', (NSLOT_T, H))
    d['cidx'] = din('cidx', (maxM, NW), dt.int32)
    d['lidx'] = din('lidx', (maxM, NW), dt.int32)
    d['lv_maskT'] = din('lv_maskT', (NW * 128, HC * maxM))
    d['gen_bias'] = din('gen_bias', (NW * maxM, 4 * H))
    d['mgather_idx'] = din('mgather_idx', (maxMM, 2 * NRX), dt.int32)
    d['m_ac'] = din('m_ac', (maxMM, NRX), F32)
    d['m_bias'] = din('m_bias', (NRX * maxMM, 2 * H))

    ops_out_d = nc.dram_tensor('ops_out', [NUM_START, NQ], F32, kind="ExternalOutput")
    ns_out_d = nc.dram_tensor('ns_out', [BL, NPL * O], F32, kind="ExternalOutput")
    arena_c = nc.dram_tensor('arena_c', [NSLOT_C, H], BF)
    arena_t = nc.dram_tensor('arena_t', [NSLOT_T, H], BF)

    with tile.TileContext(nc) as tc:
        with (
            tc.tile_pool(name="cst", bufs=1) as cst,
            tc.tile_pool(name="mwk", bufs=1) as mwk,
            tc.tile_pool(name="ps_pre", bufs=2, space="PSUM") as ps_pre,
            tc.tile_pool(name="ps_tr", bufs=1, space="PSUM") as ps_tr,
            tc.tile_pool(name="ps_pred", bufs=1, space="PSUM") as ps_pred,
            tc.tile_pool(name="ps_sc", bufs=1, space="PSUM") as ps_sc,
            tc.tile_pool(name="ps_gen", bufs=2, space="PSUM") as ps_gen,
        ):
            # ---------- constants ----------
            def load_wT(nm, K, N):
                kc = K // 128
                tl = cst.tile([128, kc * N], BF, tag=nm)
                v = tl[:].rearrange("p (k n) -> p k n", k=kc, n=N)
                for k in range(kc):
                    nc.sync.dma_start(v[:, k, :], d[nm][k * 128:(k + 1) * 128, :])
                return v

            pre0_cm = tc.tile_pool(name="pre0", bufs=1)
            pre0 = pre0_cm.__enter__()
            W_ei = load_wT('W_ei', DE, H)
            WA2 = load_wT('WA2', H, H)
            Wq = load_wT('Wq', H, H)
            W_cl = load_wT('W_cl', H, H)
            W_clg = load_wT('W_clg', H, H)
            W_cr = load_wT('W_cr', 2 * H, H)
            W_crg = load_wT('W_crg', 2 * H, H)
            Wg = load_wT('Wg', 2 * H, 4 * H)
            Wm = load_wT('Wm', 2 * H, 2 * H)
            Ws1 = load_wT('Ws1', 2 * H, H)
            W_ops = load_wT('W_ops', 2 * H, NUM_START)
            kcE = DE // 128
            encT_t = pre0.tile([128, kcE * BL * S], BF, tag='encT')
            encT = encT_t[:].rearrange("p (k n) -> p k n", k=kcE, n=BL * S)
            for k in range(kcE):
                nc.sync.dma_start(encT[:, k, :], d['encT'][k * 128:(k + 1) * 128, :])

            def load_plain(nm, shape, dty=BF):
                tl = cst.tile(list(shape), dty, tag=nm)
                nc.gpsimd.dma_start(tl[:], d[nm][:])
                return tl

            ident = load_plain('identity', (128, 128))
            ident_f = load_plain('identity_f', (128, 128), F32)
            vT = load_plain('vT', (128, HC * BL * BL))
            vTv = vT[:].rearrange("p (c l j) -> p c l j", c=HC, l=BL, j=BL)
            vsT = load_plain('vsT', (128, HC * BL * BL))
            vsTv = vsT[:].rearrange("p (c l j) -> p c l j", c=HC, l=BL, j=BL)
            nums_preT = load_plain('nums_preT', (128, HC * BL * O))
            numsv = nums_preT[:].rearrange("p (c l o) -> p c l o", c=HC, l=BL, o=O)
            mask_bias = load_plain('mask_bias', (BL, S), F32)
            pred_biasT = load_plain('pred_biasT', (128, 4 * HC), F32)
            pbv = pred_biasT[:].rearrange("p (a c) -> p a c", a=4, c=HC)
            b_ops = load_plain('b_ops', (NUM_START, 1), F32)
            A_biasT = load_plain('A_biasT', (128, HC), F32)
            b_ei_T = load_plain('b_ei_T', (128, HC), F32)
            b_ei_row = load_plain('b_ei_row', (1, H))
            cidx = load_plain('cidx', (maxM, NW), dt.int32)
            lidx = load_plain('lidx', (maxM, NW), dt.int32)
            mgidx = load_plain('mgather_idx', (maxMM, 2 * NRX), dt.int32)
            m_ac = load_plain('m_ac', (maxMM, NRX), F32)
            ones_row = cst.tile([1, 128], BF, tag='ones_row')
            nc.vector.memset(ones_row[:], 1.0)

            # arena inits: full blobs staged through SBUF, <=128 rows at a time
            def arena_init(dram_arena, init_d, n_slot, tag):
                for ofs in range(0, n_slot, 128):
                    n = min(128, n_slot - ofs)
                    tl = pre0.tile([n, H], BF, tag=tag)
                    nc.sync.dma_start(tl[:, :], init_d[ofs:ofs + n, :])
                    nc.gpsimd.dma_start(dram_arena[ofs:ofs + n, :], tl[:, :])
            arena_init(arena_c, d['arena_c_init'], NSLOT_C, 'ainitc')
            arena_init(arena_t, d['arena_t_init'], NSLOT_T, 'ainitt')

            # ---------- merge round ----------
            def emit_merge(L):
                MMr = int(MM[L])
                MMg = max(MMr, 2)  # indirect DMA needs >= 2 offsets
                tm_rows = mwk.tile([MMg, H], BF, tag='mrows')
                nc.gpsimd.indirect_dma_start(
                    out=tm_rows[:, :], out_offset=None, in_=arena_t[:],
                    in_offset=bass.IndirectOffsetOnAxis(ap=mgidx[:MMg, L - 1:L], axis=0))
                cur_rows = mwk.tile([MMg, H], BF, tag='mcur')
                nc.gpsimd.indirect_dma_start(
                    out=cur_rows[:, :], out_offset=None, in_=arena_t[:],
                    in_offset=bass.IndirectOffsetOnAxis(ap=mgidx[:MMg, NRX + L - 1:NRX + L], axis=0))
                MMp = (MMr + 1) // 2 * 2  # even offset for bf16 alignment
                pst = ps_tr.tile([128, HC * 2 * MMp], BF, tag='tr')
                pv = pst[:].rearrange("p (c m) -> p c m", c=HC, m=2 * MMp)
                for c in range(HC):
                    nc.tensor.transpose(pv[:, c, :MMr], tm_rows[:MMr, c * 128:(c + 1) * 128],
                                        ident[:MMr, :MMr])
                    nc.tensor.transpose(pv[:, c, MMp:MMp + MMr], cur_rows[:MMr, c * 128:(c + 1) * 128],
                                        ident[:MMr, :MMr])
                miT = mwk.tile([128, HC * 2 * MMp], BF, tag='miT')
                miv = miT[:].rearrange("p (c m) -> p c m", c=HC, m=2 * MMp)
                nc.scalar.activation(miv[:, :, :MMr], pv[:, :, :MMr], AF.Copy)
                nc.scalar.activation(miv[:, :, MMp:MMp + MMr], pv[:, :, MMp:MMp + MMr], AF.Copy)
                mb = mwk.tile([MMr, 2 * H], BF, tag='mb')
                nc.sync.dma_start(mb[:, :], d['m_bias'][(L - 1) * maxMM:(L - 1) * maxMM + MMr, :])
                pm = ps_gen.tile([MMr, H], F32, tag='gen')
                pmg = ps_gen.tile([MMr, H], F32, tag='gen')
                for half, ps in ((0, pm), (1, pmg)):
                    nc.tensor.matmul(ps[:, :], ident[:MMr, :MMr],
                                     mb[:, half * H:(half + 1) * H], start=True, stop=False)
                    for k in range(2 * HC):
                        lhs = (miv[:, k, :MMr] if k < HC
                               else miv[:, k - HC, MMp:MMp + MMr])
                        nc.tensor.matmul(ps[:, :], lhs, Wm[:, k, half * H:(half + 1) * H],
                                         start=False, stop=(k == 2 * HC - 1))
                ta = mwk.tile([MMr, H], BF, tag='mta')
                tg = mwk.tile([MMr, H], BF, tag='mtg')
                nc.scalar.activation(ta[:, :], pm[:, :], AF.Tanh)
                nc.scalar.activation(tg[:, :], pmg[:, :], AF.Tanh, scale=0.5)
                sg = mwk.tile([MMr, H], BF, tag='msg')
                nc.vector.tensor_scalar(sg[:, :], tg[:, :], 0.5, 0.5, ALU.mult, ALU.add)
                gate = mwk.tile([MMr, H], BF, tag='mgate')
                nc.vector.tensor_tensor(gate[:, :], ta[:, :], sg[:, :], op=ALU.mult)
                diff = mwk.tile([MMr, H], BF, tag='mdiff')
                nc.vector.tensor_tensor(diff[:, :], gate[:, :], cur_rows[:MMr, :], op=ALU.subtract)
                nc.vector.tensor_scalar(diff[:, :], diff[:, :],
                                        m_ac[:MMr, L - 1:L], None, ALU.mult)
                new = mwk.tile([MMr, H], BF, tag='mnew')
                nc.vector.tensor_tensor(new[:, :], diff[:, :], cur_rows[:MMr, :], op=ALU.add)
                nc.gpsimd.dma_start(arena_t[round_base[L]:round_base[L] + MMr, :], new[:, :])

            if NR >= 1:
                emit_merge(1)
            if NR >= 2:
                emit_merge(2)

            # ---------- precompute ----------
            embT = pre0.tile([128, HC * BL * S], BF, tag='embT')
            embTv = embT[:].rearrange("p (c f) -> p c f", c=HC, f=BL * S)
            A_sb = cst.tile([128, HC * BL * S], BF, tag='A_sb')
            A_f = A_sb[:].rearrange("p (c f) -> p c f", c=HC, f=BL * S)
            A_v = A_sb[:].rearrange("p (c l s) -> p c l s", c=HC, l=BL, s=S)
            emb_s = cst.tile([128, 2 * BL * H], BF, tag='emb_s')
            emb_sv = emb_s[:].rearrange("p (sc l h) -> p sc l h", sc=2, l=BL, h=H)

            FC = BL * S // 512
            for hc in range(HC):
                for fc in range(FC):
                    ps = ps_pre.tile([128, 512], F32, tag='pre')
                    for k in range(KE):
                        nc.tensor.matmul(ps[:], W_ei[:, k, hc * 128:(hc + 1) * 128],
                                         encT[:, k, fc * 512:(fc + 1) * 512],
                                         start=(k == 0), stop=(k == KE - 1))
                    nc.scalar.activation(embTv[:, hc, fc * 512:(fc + 1) * 512], ps[:],
                                         AF.Tanh, bias=b_ei_T[:, hc:hc + 1])
            for hc in range(HC):
                for fc in range(FC):
                    ps = ps_pre.tile([128, 512], F32, tag='pre')
                    for k in range(HC):
                        nc.tensor.matmul(ps[:], WA2[:, k, hc * 128:(hc + 1) * 128],
                                         embTv[:, k, fc * 512:(fc + 1) * 512],
                                         start=(k == 0), stop=(k == HC - 1))
                    nc.scalar.activation(A_f[:, hc, fc * 512:(fc + 1) * 512], ps[:],
                                         AF.Identity, bias=A_biasT[:, hc:hc + 1])
            for lane in range(BL):
                for sc in range(2):
                    ps = ps_pre.tile([128, 512], F32, tag='pre')
                    nc.tensor.matmul(ps[:], ones_row[:], b_ei_row[:],
                                     start=True, stop=False)
                    for k in range(KE):
                        nc.tensor.matmul(
                            ps[:],
                            encT[:, k, lane * S + sc * 128: lane * S + (sc + 1) * 128],
                            W_ei[:, k, :], start=False, stop=(k == KE - 1))
                    nc.scalar.activation(emb_sv[:, sc, lane, :], ps[:], AF.Tanh)

            pre0_cm.__exit__(None, None, None)
            wk_cm = tc.tile_pool(name="wk", bufs=2)
            wk = wk_cm.__enter__()
            ep_cm = tc.tile_pool(name="ep", bufs=3)
            ep = ep_cm.__enter__()
            NQl = BL * NPL
            qbufW = cst.tile([128, HC * NQl], BF, tag='qbufW')
            qW = qbufW[:].rearrange("p (c q) -> p c q", c=HC, q=NQl)
            cbufW = cst.tile([128, HC * NQl], BF, tag='cbufW')
            cW = cbufW[:].rearrange("p (c q) -> p c q", c=HC, q=NQl)
            qbufL = cst.tile([128, HC * NQl], BF, tag='qbufL')
            qvL = qbufL[:].rearrange("p (c l n) -> p c l n", c=HC, l=BL, n=NPL)
            qL = qbufL[:].rearrange("p (c q) -> p c q", c=HC, q=NQl)
            cbufL = cst.tile([128, HC * NQl], BF, tag='cbufL')
            cvL = cbufL[:].rearrange("p (c l n) -> p c l n", c=HC, l=BL, n=NPL)
            cL = cbufL[:].rearrange("p (c q) -> p c q", c=HC, q=NQl)

            # ---------- wave ----------
            def emit_wave(w):
                Mw, Gw = int(M[w]), int(G[w])
                off = int(goff[w])
                # gathers
                c_rows = wk.tile([Mw, H], BF, tag='crows')
                nc.gpsimd.indirect_dma_start(
                    out=c_rows[:, :], out_offset=None, in_=arena_c[:],
                    in_offset=bass.IndirectOffsetOnAxis(ap=cidx[:Mw, w:w + 1], axis=0))
                l_rows = wk.tile([Mw, H], BF, tag='lrows')
                nc.gpsimd.indirect_dma_start(
                    out=l_rows[:, :], out_offset=None, in_=arena_t[:],
                    in_offset=bass.IndirectOffsetOnAxis(ap=lidx[:Mw, w:w + 1], axis=0))
                # transpose c and l -> clT (128, [c, 2Mw]): cols :Mw = cT, Mw: = lT
                pst = ps_tr.tile([128, HC * 2 * Mw], BF, tag='tr')
                pv = pst[:].rearrange("p (c m) -> p c m", c=HC, m=2 * Mw)
                for c in range(HC):
                    nc.tensor.transpose(pv[:, c, :Mw], c_rows[:, c * 128:(c + 1) * 128],
                                        ident[:Mw, :Mw])
                    nc.tensor.transpose(pv[:, c, Mw:], l_rows[:, c * 128:(c + 1) * 128],
                                        ident[:Mw, :Mw])
                clT = wk.tile([128, HC * 2 * Mw], BF, tag='clT')
                clv = clT[:].rearrange("p (c m) -> p c m", c=HC, m=2 * Mw)
                nc.scalar.activation(clT[:, :], pst[:, :], AF.Copy)

                # pred args psum: [gl, glg, gr, grg] packed (128, [4, HC, Mw])
                pp = ps_pred.tile([128, 5 * HC * Mw], F32, tag='pp')
                ppv = pp[:].rearrange("p (a c m) -> p a c m", a=5, c=HC, m=Mw)
                specs = [(W_cl, HC, False), (W_clg, HC, False),
                         (W_cr, 2 * HC, True), (W_crg, 2 * HC, True)]
                for a, (W_, kcnt, uses_l) in enumerate(specs):
                    for hc in range(HC):
                        for k in range(kcnt):
                            rhs = clv[:, k, :Mw] if k < HC else clv[:, k - HC, Mw:]
                            nc.tensor.matmul(ppv[:, a, hc, :], W_[:, k, hc * 128:(hc + 1) * 128],
                                             rhs, start=(k == 0), stop=(k == kcnt - 1))
                # gate nonlinearities (T-layout, bf16)
                glt = wk.tile([128, HC * Mw], BF, tag='glt')
                glg = wk.tile([128, HC * Mw], BF, tag='glg')
                grt = wk.tile([128, HC * Mw], BF, tag='grt')
                grg = wk.tile([128, HC * Mw], BF, tag='grg')
                if pred_bias_zero:
                    nc.scalar.activation(glt[:, :], pp[:, 0 * HC * Mw:1 * HC * Mw], AF.Tanh)
                    nc.scalar.activation(glg[:, :], pp[:, 1 * HC * Mw:2 * HC * Mw], AF.Tanh, scale=0.5)
                    nc.scalar.activation(grt[:, :], pp[:, 2 * HC * Mw:3 * HC * Mw], AF.Tanh)
                    nc.scalar.activation(grg[:, :], pp[:, 3 * HC * Mw:4 * HC * Mw], AF.Tanh, scale=0.5)
                else:
                    for a, dst, sc_ in ((0, glt, 1.0), (1, glg, 0.5), (2, grt, 1.0), (3, grg, 0.5)):
                        dv = dst[:].rearrange("p (c m) -> p c m", c=HC, m=Mw)
                        for hc in range(HC):
                            nc.scalar.activation(dv[:, hc, :], ppv[:, a, hc, :], AF.Tanh,
                                                 bias=pbv[:, a, hc:hc + 1], scale=sc_)
                # sigma and gate products
                nc.vector.tensor_scalar(glg[:, :], glg[:, :], 0.5, 0.5, ALU.mult, ALU.add)
                nc.vector.tensor_scalar(grg[:, :], grg[:, :], 0.5, 0.5, ALU.mult, ALU.add)
                nc.vector.tensor_tensor(glt[:, :], glt[:, :], glg[:, :], op=ALU.mult)
                nc.vector.tensor_tensor(grt[:, :], grt[:, :], grg[:, :], op=ALU.mult)
                # q select -> qbuf slice
                lvm = wk.tile([128, HC * maxM], BF, tag='lvm')
                nc.sync.dma_start(lvm[:, :], d['lv_maskT'][w * 128:(w + 1) * 128, :])
                nc.vector.tensor_tensor(grt[:, :], grt[:, :], glt[:, :], op=ALU.subtract)
                nc.vector.tensor_tensor(grt[:, :], grt[:, :], lvm[:, :HC * Mw], op=ALU.mult)
                qb = BL * off  # wave-major column base
                qsl = qW[:, :, qb:qb + Mw]
                nc.vector.tensor_tensor(
                    qsl, grt[:].rearrange("p (c m) -> p c m", c=HC, m=Mw),
                    glt[:].rearrange("p (c m) -> p c m", c=HC, m=Mw), op=ALU.add)
                nc.vector.tensor_copy(
                    qvL[:, :, :, off:off + Gw],
                    qW[:, :, qb:qb + Mw].rearrange("p c (l g) -> p c l g", l=BL, g=Gw))

                # u = Wq^T q (into the a=4 slice of the pred bank)
                for hc in range(HC):
                    for k in range(HC):
                        nc.tensor.matmul(ppv[:, 4, hc, :], Wq[:, k, hc * 128:(hc + 1) * 128],
                                         qW[:, k, qb:qb + Mw],
                                         start=(k == 0), stop=(k == HC - 1))
                uT = wk.tile([128, HC * Mw], BF, tag='uT')
                nc.scalar.activation(uT[:, :], pp[:, 4 * HC * Mw:5 * HC * Mw], AF.Copy)
                uTv = uT[:].rearrange("p (c l g) -> p c l g", c=HC, l=BL, g=Gw)

                # attention
                psc = ps_sc.tile([BL, Gw * S], F32, tag='sc')
                for lane in range(BL):
                    e = ep.tile([128, HC * Gw * S], BF, tag='e')
                    ev = e[:].rearrange("p (c g s) -> p c g s", c=HC, g=Gw, s=S)
                    nc.vector.tensor_tensor(
                        ev,
                        A_v[:, :, lane:lane + 1, :].to_broadcast([128, HC, Gw, S]),
                        uTv[:, :, lane, :].unsqueeze(3).to_broadcast([128, HC, Gw, S]),
                        op=ALU.add)
                    nc.scalar.activation(e[:, :], e[:, :], AF.Tanh)
                    for c in range(HC):
                        nc.tensor.matmul(psc[:, :], vTv[:, c, lane, :],
                                         e[:, c * Gw * S:(c + 1) * Gw * S],
                                         start=(lane == 0 and c == 0),
                                         stop=(lane == BL - 1 and c == HC - 1))
                # softmax (no max-sub; scores bounded by |v|_1)
                sc_sb = wk.tile([BL, Gw * S], F32, tag='sc_sb')
                pscv = psc[:].rearrange("p (g s) -> p g s", g=Gw, s=S)
                nc.vector.tensor_tensor(
                    sc_sb[:].rearrange("p (g s) -> p g s", g=Gw, s=S),
                    pscv, mask_bias[:].unsqueeze(1).to_broadcast([BL, Gw, S]), op=ALU.add)
                ex = wk.tile([BL, Gw * S], F32, tag='ex')
                nc.scalar.activation(ex[:, :], sc_sb[:, :], AF.Exp, bias=float(bv_attn))
                sums = wk.tile([BL, Gw], F32, tag='sums')
                nc.vector.reduce_sum(sums[:, :], ex[:].rearrange("p (g s) -> p g s", g=Gw, s=S),
                                     axis=AX.X)
                rec = wk.tile([BL, Gw], F32, tag='rec')
                nc.vector.reciprocal(rec[:, :], sums[:, :])
                a_f = wk.tile([BL, Gw * S], F32, tag='a_f')
                nc.vector.tensor_tensor(
                    a_f[:].rearrange("p (g s) -> p g s", g=Gw, s=S),
                    ex[:].rearrange("p (g s) -> p g s", g=Gw, s=S),
                    rec[:].unsqueeze(2).to_broadcast([BL, Gw, S]), op=ALU.mult)
                # a transpose (f32): (BL, [g, sc*128]) -> psum (128, [sc, g, BL]);
                # copy to bf16 aT in [sc, l, g] layout (even offsets for bf16)
                pat = ps_tr.tile([128, 2 * Gw * BL], F32, tag='atr')
                patv = pat[:].rearrange("p (sc g l) -> p sc g l", sc=2, g=Gw, l=BL)
                abv = a_f[:].rearrange("p (g sc f) -> p g sc f", g=Gw, sc=2, f=128)
                for g in range(Gw):
                    for sc_i in range(2):
                        nc.tensor.transpose(patv[:, sc_i, g, :], abv[:, g, sc_i, :],
                                            ident_f[:BL, :BL])
                aT = wk.tile([128, 2 * BL * Gw], BF, tag='aT')
                aTv = aT[:].rearrange("p (sc l g) -> p sc l g", sc=2, l=BL, g=Gw)
                nc.scalar.activation(aTv, patv.transpose([0, 1, 3, 2]), AF.Copy)
                # ctx per lane
                ctx_rows = wk.tile([Mw, H], BF, tag='ctx_rows')
                for lane in range(BL):
                    pc = ps_gen.tile([Gw, H], F32, tag='gen')
                    for sc_i in range(2):
                        nc.tensor.matmul(pc[:, :], aTv[:, sc_i, lane, :],
                                         emb_sv[:, sc_i, lane, :],
                                         start=(sc_i == 0), stop=(sc_i == 1))
                    cl_sb = wk.tile([Gw, H], BF, tag='cl_sb')
                    nc.scalar.activation(cl_sb[:, :], pc[:, :], AF.Copy)
                    nc.sync.dma_start(ctx_rows[lane * Gw:(lane + 1) * Gw, :], cl_sb[:, :])
                # ctx transpose -> cbuf slice
                pct = ps_tr.tile([128, HC * Mw], BF, tag='tr')
                pctv = pct[:].rearrange("p (c m) -> p c m", c=HC, m=Mw)
                for c in range(HC):
                    nc.tensor.transpose(pctv[:, c, :], ctx_rows[:, c * 128:(c + 1) * 128],
                                        ident[:Mw, :Mw])
                nc.scalar.activation(cW[:, :, qb:qb + Mw], pctv, AF.Copy)
                nc.vector.tensor_copy(
                    cvL[:, :, :, off:off + Gw],
                    cW[:, :, qb:qb + Mw].rearrange("p c (l g) -> p c l g", l=BL, g=Gw))

                # gen: lhsT = [qT; ctxT] chunks, rhs = Wg columns; + host bias rows
                gb = wk.tile([Mw, 4 * H], BF, tag='gb')
                nc.sync.dma_start(gb[:, :], d['gen_bias'][w * maxM:w * maxM + Mw, :])
                for sd in range(2):
                    pgs = []
                    for jj in range(2):
                        j = 2 * sd + jj
                        pg = ps_gen.tile([Mw, H], F32, tag='gen')
                        nc.tensor.matmul(pg[:, :], ident[:Mw, :Mw], gb[:, j * H:(j + 1) * H],
                                         start=True, stop=False)
                        for k in range(2 * HC):
                            lhs = (qW[:, k, qb:qb + Mw] if k < HC
                                   else cW[:, k - HC, qb:qb + Mw])
                            nc.tensor.matmul(pg[:, :], lhs, Wg[:, k, j * H:(j + 1) * H],
                                             start=False, stop=(k == 2 * HC - 1))
                        pgs.append(pg)
                    ta = wk.tile([Mw, H], BF, tag='gta')
                    tg = wk.tile([Mw, H], BF, tag='gtg')
                    nc.scalar.activation(ta[:, :], pgs[0][:, :], AF.Tanh)
                    nc.scalar.activation(tg[:, :], pgs[1][:, :], AF.Tanh, scale=0.5)
                    nc.vector.tensor_scalar(tg[:, :], tg[:, :], 0.5, 0.5, ALU.mult, ALU.add)
                    cand = wk.tile([Mw, H], BF, tag='cand')
                    nc.vector.tensor_tensor(cand[:, :], ta[:, :], tg[:, :], op=ALU.mult)
                    base = cand_base[w] + sd * Mw
                    nc.gpsimd.dma_start(arena_c[base:base + Mw, :], cand[:, :])

            # ---------- interleaved emission ----------
            for w in range(NW):
                emit_wave(w)
                L = w + 3
                if L <= NR:
                    emit_merge(L)

            # ---------- end batch: outputs ----------
            uws = wk.tile([128, HC * NQ], BF, tag='uws')
            uwsv = uws[:].rearrange("p (c l n) -> p c l n", c=HC, l=BL, n=NPL)
            for hc in range(HC):
                ps = ps_pre.tile([128, NQ], F32, tag='pre')
                for k in range(2 * HC):
                    rhs = qL[:, k, :] if k < HC else cL[:, k - HC, :]
                    nc.tensor.matmul(ps[:, :], Ws1[:, k, hc * 128:(hc + 1) * 128],
                                     rhs, start=(k == 0), stop=(k == 2 * HC - 1))
                nc.scalar.activation(uws[:, hc * NQ:(hc + 1) * NQ], ps[:, :], AF.Copy)
            pns = ps_sc.tile([BL, NPL * O], F32, tag='sc')
            for lane in range(BL):
                se = ep.tile([128, HC * NPL * O], BF, tag='se')
                sev = se[:].rearrange("p (c n o) -> p c n o", c=HC, n=NPL, o=O)
                nc.vector.tensor_tensor(
                    sev,
                    uwsv[:, :, lane, :].unsqueeze(3).to_broadcast([128, HC, NPL, O]),
                    numsv[:, :, lane, :].unsqueeze(2).to_broadcast([128, HC, NPL, O]),
                    op=ALU.add)
                nc.scalar.activation(se[:, :], se[:, :], AF.Tanh)
                for c in range(HC):
                    nc.tensor.matmul(pns[:, :], vsTv[:, c, lane, :],
                                     se[:, c * NPL * O:(c + 1) * NPL * O],
                                     start=(lane == 0 and c == 0),
                                     stop=(lane == BL - 1 and c == HC - 1))
            ns_sb = wk.tile([BL, NPL * O], F32, tag='ns_sb')
            nc.scalar.activation(ns_sb[:, :], pns[:, :], AF.Identity, bias=float(bv_score))
            nc.sync.dma_start(ns_out_d[:, :], ns_sb[:, :])

            pop = ps_sc.tile([NUM_START, NQ], F32, tag='sc')
            for k in range(2 * HC):
                rhs = qL[:, k, :] if k < HC else cL[:, k - HC, :]
                nc.tensor.matmul(pop[:, :], W_ops[:, k, :], rhs,
                                 start=(k == 0), stop=(k == 2 * HC - 1))
            ops_sb = wk.tile([NUM_START, NQ], F32, tag='ops_sb')
            nc.scalar.activation(ops_sb[:, :], pop[:, :], AF.Identity, bias=b_ops[:, :])
            nc.sync.dma_start(ops_out_d[:, :], ops_sb[:, :])
            ep_cm.__exit__(None, None, None)
            wk_cm.__exit__(None, None, None)

    nc.compile()
    return nc


# ============================ entry point ==================================


def _ensure_ntff_hook():
    """Provide antenv.axon_hooks (NTFF profiling) if the image lacks it."""
    import types
    import contextlib
    import ctypes
    if 'antenv.axon_hooks' in sys.modules:
        return
    try:
        import antenv
    except ImportError:
        return
    so_path = '/opt/axon/libaxon_pjrt.so'
    holder = {'h': None}
    if os.path.exists(so_path):
        try:
            lib = ctypes.CDLL(so_path)
            if hasattr(lib, 'axon_start_nrt_profile'):
                lib.axon_start_nrt_profile.argtypes = [
                    ctypes.POINTER(ctypes.c_int64), ctypes.c_size_t]
                lib.axon_start_nrt_profile.restype = ctypes.c_int64
                lib.axon_stop_nrt_profile.argtypes = [ctypes.c_char_p]
                lib.axon_stop_nrt_profile.restype = ctypes.c_int64

                @contextlib.contextmanager
                def _hook(output_dir, device_ids):
                    import jax
                    jax.devices()
                    if device_ids:
                        ids = (ctypes.c_int64 * len(device_ids))(*device_ids)
                        rc = lib.axon_start_nrt_profile(ids, len(device_ids))
                    else:
                        rc = lib.axon_start_nrt_profile(None, 0)
                    if rc != 0:
                        raise RuntimeError(f"axon_start_nrt_profile rc={rc}")
                    try:
                        yield
                    finally:
                        n = lib.axon_stop_nrt_profile(str(output_dir).encode())
                        print(f"profile: {n} file(s) written to {output_dir}")
                holder['h'] = _hook
        except OSError:
            pass
    mod = types.ModuleType('antenv.axon_hooks')
    mod.set_axon_ntff_profile_hook = lambda h: holder.__setitem__('h', h)
    mod.get_axon_ntff_profile_hook = lambda: holder['h']
    sys.modules['antenv.axon_hooks'] = mod
    antenv.axon_hooks = mod


_CACHE = {}
TRACE = False
LAST_EXEC_NS = None
LAST_RESULT = None


def kernel(x, y, xnm, xnp, encoder_embed, encoder_summary, params):
    sch = make_schedule(y)
    meta = sched_meta(sch, params)
    in_maps, hosts = [], []
    for core in range(NCORE):
        inp, host = host_prep(sch, core, x, xnm, xnp, encoder_embed,
                              encoder_summary, params)
        in_maps.append(inp)
        hosts.append(host)

    key = repr(sorted(meta.items(), key=lambda kv: kv[0]))
    if key not in _CACHE:
        _CACHE[key] = emit_program(meta)
    nc = _CACHE[key]

    from concourse.bass_utils import run_bass_kernel_spmd
    if TRACE:
        _ensure_ntff_hook()
    global LAST_EXEC_NS, LAST_RESULT
    res = run_bass_kernel_spmd(nc, in_maps, list(range(NCORE)), trace=TRACE, tmpdir=os.environ.get('BASS_TMPDIR'))
    LAST_EXEC_NS = res.exec_time_ns
    LAST_RESULT = res
    core_outs = res.results
    return assemble_output(sch, core_outs, hosts)
